# revision 39
# baseline (speedup 1.0000x reference)
"""LoRA linear (dropout -> x @ A.T @ B.T * scaling) on 8 TRN2 NeuronCores.

Data-parallel over tokens: each core handles T/8 = 2048 tokens; lora_A/lora_B
are replicated.

Precision/traffic plan: x and out travel as bf16; the dropout decision ships
as a 1-BIT mask packed into u16 bit-planes (1MB/core instead of 8MB fp8).
Per-core HBM traffic: 16MB x + 1MB u + 16MB out = 33MB (first baseline moved
40MB). The mask is exact (host computes drop_u >= 0.1, packs bits), so the
only error is bf16 rounding (~3.5e-3 rel vs the 2e-2 gate).

On-device mask expansion: plane j of a block is extracted with a single
bitVec tensor_scalar (mt = u16 & (1<<j), u16->u16; bitVec ops cannot cast so
the mask stays u16). The mask values are {0, 2^j}; the 2^j is compensated
exactly by pre-scaling the packed lora_A chunk kc by 2^-(kc//2) on host
(power-of-two scales are exact in bf16). The arith tensor_tensor mult
xd = x * mask casts u16->f32 implicitly. The dropout 1/(1-p) and alpha/r
scalings are folded into lora_B on host. All elementwise work runs on DVE:
GPSIMD's Pool engine rejects tensor_scalar bitVec ops, and its
tensor_tensor measured 3ns/elem AND its SBUF-port contention slowed
concurrent DVE extractions 1.8x — a strict loss.

PE: mm1 runs column-tiled — two concurrent M=64 matmuls at tile_position
(0,0)/(0,64) accumulating even/odd kc chunks into PSUM partitions 0-63 /
64-127 — halving mm1 stream cycles. The even/odd halves are summed for free
inside mm2 by replicating B^T onto PSUM partitions 64-127 (mm2 contracts
K=128 instead of 64). A ~60ns dependency-free "heartbeat" matmul is dropped
into the PE queue before each potentially-stalling burst so the HAM clock
gate stays at K=8/8 (2.4GHz) through DMA/DVE waits (cut throttle time
71us -> 34us).

Schedule: per block the pending mm2 half from the previous block runs at
the HEAD of the iteration (its inputs are ready; queuing it after this
block's mm1 would stall the PE and starve ACT's copies behind DVE).
PSUM->SBUF out copies run on ACT except the drain blocks (>= 6) where DVE
is free and they alternate 2/2.

Layout: host packs x into the exact transposed SBUF tile layout (per
256-token block: [128 i-partitions, 32 i-chunks x 256 tokens]) so loads are
16KB-contiguous rows; u is one [128, NB*512] u16 tensor loaded once on the
(initially idle) scalar queue. Loads on sync queue, stores on scalar queue.
Host upcasts bf16 out to fp32.

HW-measured schedule notes (do not re-try): per-block load splitting, a
single 2MB store per block (o as [128, NB*2*OUT]), osb bufs=2, xpool
bufs=5, and GPSIMD offload all regress 3-10us.
"""

import sys

sys.path.insert(0, "/opt/trn_rl_repo")

import ml_dtypes
import numpy as np

import concourse.bacc as bacc
import concourse.tile as tile
from concourse import mybir
from concourse.bass_utils import run_bass_kernel_spmd

N_CORES = 8
T, IN, OUT, R = 16384, 4096, 4096, 64
TS = T // N_CORES  # tokens per core (2048)
P_DROP = 0.1
SCALE = (128.0 / 64.0) / (1.0 - P_DROP)  # alpha/r * 1/(1-p), folded into B

F32 = mybir.dt.float32
BF16 = mybir.dt.bfloat16
U16 = mybir.dt.uint16
NPBF16 = np.dtype(ml_dtypes.bfloat16)

KC = IN // 128  # contraction chunks (32)
TB = 128  # tokens per block
NB = TS // TB  # blocks per core (16)
W = KC * TB  # packed row width (4096)
NPL = 16  # bit planes per block (u16)
PW = W // NPL  # columns per plane (256)
NCH = 2  # column chunks per block
CCW = W // NCH
CKC = KC // NCH
DRAIN = NB - 4  # blocks whose copies split ACT/DVE 2/2


def _emit(tc, x, u, a, b, o):
    """Per-core program. x is [NB*128, W] packed transposed blocks with
    element (blk*128+p, kc*TB+t) = x[blk*TB+t, kc*128+p]. u is [128, NB*PW]
    u16 bit-planes: bit j of u[p, blk*PW+c] is the keep bit for packed
    column j*PW+c of block blk. a is [128, KC*64] packed A chunks
    (a[p, kc*64+r] = A[r, kc*128+p] * 2^-(kc//2)), b is [64, OUT] scaled B
    transposed (replicated on-chip to 128 partitions), o is [TS, OUT]."""
    nc = tc.nc
    from contextlib import ExitStack

    with ExitStack() as ctx:
        const = ctx.enter_context(tc.tile_pool(name="const", bufs=1))
        xpool = ctx.enter_context(tc.tile_pool(name="xp", bufs=8))
        mpool = ctx.enter_context(tc.tile_pool(name="mp", bufs=2))
        hpool = ctx.enter_context(tc.tile_pool(name="hp", bufs=2))
        opool = ctx.enter_context(tc.tile_pool(name="op", bufs=3))
        # psh=1: ph(k) is drained by the hT copy during mm2(k,h0), well
        # before mm1(k+1) reaches the head of the in-order PE queue.
        psh = ctx.enter_context(tc.tile_pool(name="psh", bufs=1, space="PSUM"))
        pso = ctx.enter_context(tc.tile_pool(name="pso", bufs=3, space="PSUM"))
        psw = ctx.enter_context(tc.tile_pool(name="psw", bufs=1, space="PSUM"))

        u_sb = const.tile([128, NB * PW], U16)
        nc.scalar.dma_start(u_sb[:], u[:, :])
        a_sb = const.tile([128, KC * R], BF16)
        nc.scalar.dma_start(a_sb[:], a[:, :])
        b_sb = const.tile([128, OUT], BF16)  # B^T replicated twice
        nc.scalar.dma_start(b_sb[0:R, :], b[:, :])
        nc.scalar.dma_start(b_sb[R : 2 * R, :], b[:, :])

        hb_ps = psw.tile([R, 64], F32)

        def _beat():
            nc.tensor.matmul(
                hb_ps[:], a_sb[:, 0:R], a_sb[:, 0:64], start=True, stop=True,
                skip_group_check=True,
            )

        def _mm2(blk, hT, tail):
            # full 128-token block of out = hT.T @ b_sb (K=128 contracts
            # the even/odd kc halves): 8 matmuls + cast-copies out of PSUM
            # + one 1MB store.
            osb = opool.tile([128, OUT], BF16)
            for g in range(OUT // 1024):
                po = pso.tile([128, 1024], F32, tag="po")
                for j in range(2):
                    oc = g * 2 + j
                    nc.tensor.matmul(
                        po[:, j * 512 : (j + 1) * 512],
                        hT[:],
                        b_sb[:, oc * 512 : (oc + 1) * 512],
                        start=True,
                        stop=True,
                    )
                dst = osb[:, g * 1024 : (g + 1) * 1024]
                if tail and g % 2 == 1:
                    nc.vector.tensor_copy(dst, po[:])
                else:
                    nc.scalar.copy(dst, po[:])
            nc.scalar.dma_start(o[blk * TB : (blk + 1) * TB, :], osb[:])

        pending = None
        for blk in range(NB):
            rows = slice(blk * 128, (blk + 1) * 128)
            xt = xpool.tile([128, W], BF16)
            nc.sync.dma_start(xt[:], x[rows, :])
            # the pending mm2 half's inputs are ready NOW — run it at the
            # head of the block so the PE (and then ACT's copies) have
            # guaranteed work while DVE masks this block.
            if pending is not None:
                _beat()
                _mm2(*pending, pending[0] >= DRAIN)
                pending = None
            us = u_sb[:, blk * PW : (blk + 1) * PW]
            mt = mpool.tile([128, W], U16)
            ph = psh.tile([128, TB], F32)

            def _ext(pl):
                nc.vector.tensor_scalar(
                    mt[:, pl * PW : (pl + 1) * PW],
                    us,
                    1 << pl,
                    None,
                    mybir.AluOpType.bitwise_and,
                )

            def _mm1(kc_pairs):
                # column-tiled: even kc -> PE cols 0-63 / PSUM 0-63,
                # odd kc -> PE cols 64-127 / PSUM 64-127, concurrent.
                for kc in kc_pairs:
                    half = kc % 2
                    nc.tensor.matmul(
                        ph[half * R : (half + 1) * R, :],
                        a_sb[:, kc * R : (kc + 1) * R],
                        xt[:, kc * TB : (kc + 1) * TB],
                        start=(kc < 2),
                        stop=(kc >= KC - 2),
                        tile_position=(0, half * R),
                    )


            for pl in range(8):
                _ext(pl)
            nc.vector.tensor_tensor(
                xt[:, 0:CCW], xt[:, 0:CCW], mt[:, 0:CCW],
                mybir.AluOpType.mult,
            )
            for pl in range(8, 16):
                _ext(pl)
            _beat()
            _mm1(range(0, CKC))
            nc.vector.tensor_tensor(
                xt[:, CCW:W], xt[:, CCW:W], mt[:, CCW:W],
                mybir.AluOpType.mult,
            )
            _beat()
            _mm1(range(CKC, KC))
            hT = hpool.tile([128, TB], BF16)
            nc.scalar.copy(hT[:], ph[:])
            pending = (blk, hT)
        _beat()
        _mm2(*pending, True)


def build_nc():
    nc = bacc.Bacc()
    x_d = nc.declare_dram_parameter("x", [NB * 128, W], BF16, isOutput=False)
    u_d = nc.declare_dram_parameter("u", [128, NB * PW], U16, isOutput=False)
    a_d = nc.declare_dram_parameter("a", [128, KC * R], BF16, isOutput=False)
    b_d = nc.declare_dram_parameter("b", [R, OUT], BF16, isOutput=False)
    o_d = nc.declare_dram_parameter("o", [TS, OUT], BF16, isOutput=True)
    with tile.TileContext(nc) as tc:
        _emit(tc, x_d[:], u_d[:], a_d[:], b_d[:], o_d[:])
    if not nc.is_finalized():
        nc.finalize()
    return nc


_NC_CACHE = None


def _get_nc():
    global _NC_CACHE
    if _NC_CACHE is None:
        _NC_CACHE = build_nc()
    return _NC_CACHE


def _pack_tokens(arr, npdt):
    """[T, IN] fp32 -> per-core [NB*128, W] packed transposed blocks:
    out[c][blk*128+p, kc*TB+t] = arr[c*TS + blk*TB + t, kc*128+p]."""
    a5 = arr.reshape(N_CORES, NB, TB, KC, 128).transpose(0, 1, 4, 3, 2)
    return np.ascontiguousarray(a5.astype(npdt)).reshape(
        N_CORES, NB * 128, W
    )


def _pack_mask(drop_u):
    """[T, IN] fp32 uniforms -> per-core [128, NB*PW] u16 bit-planes."""
    keep = np.asarray(drop_u, dtype=np.float32) >= np.float32(P_DROP)
    # packed layout [C, NB, 128, W] matching x
    k5 = keep.reshape(N_CORES, NB, TB, KC, 128).transpose(0, 1, 4, 3, 2)
    k5 = k5.reshape(N_CORES, NB, 128, NPL, PW)
    shifts = (np.uint16(1) << np.arange(NPL, dtype=np.uint16))[
        None, None, None, :, None
    ]
    u16 = (k5.astype(np.uint16) * shifts).sum(axis=3, dtype=np.uint16)
    # [C, NB, 128, PW] -> [C, 128, NB*PW]
    return np.ascontiguousarray(u16.transpose(0, 2, 1, 3)).reshape(
        N_CORES, 128, NB * PW
    )


def _in_maps(x, lora_A, lora_B, drop_u):
    xp = _pack_tokens(np.asarray(x, dtype=np.float32), NPBF16)
    up = _pack_mask(drop_u)
    # a[p, kc*64+r] = A[r, kc*128+p] * 2^-(kc//2)  (mask-plane compensation)
    a3 = (
        np.asarray(lora_A, dtype=np.float32)
        .T.reshape(KC, 128, R)
        .transpose(1, 0, 2)
    )  # [128, KC, R]
    plane_scale = (np.float32(2.0) ** -(np.arange(KC) // 2))[None, :, None]
    ap = np.ascontiguousarray((a3 * plane_scale).astype(NPBF16)).reshape(
        128, KC * R
    )
    bp = np.ascontiguousarray(
        (np.asarray(lora_B, dtype=np.float32) * np.float32(SCALE))
        .T.astype(NPBF16)
    )
    return [
        {"x": xp[c], "u": up[c], "a": ap, "b": bp} for c in range(N_CORES)
    ]


def run_spmd(x, lora_A, lora_B, drop_u, **kw):
    res = run_bass_kernel_spmd(
        _get_nc(), _in_maps(x, lora_A, lora_B, drop_u), list(range(N_CORES)), **kw
    )
    out = np.concatenate(
        [np.asarray(r["o"]).astype(np.float32) for r in res.results], axis=0
    )
    return out, res


def kernel(x, lora_A, lora_B, drop_u):
    out, _ = run_spmd(x, lora_A, lora_B, drop_u)
    return out
